# revision 35
# baseline (speedup 1.0000x reference)
"""Causal multi-head attention (B=2, L=2048, D=1024, H=16) on 8 trn2 cores.

Sharding: DP on batch (2) x TP on heads (4 groups of 4 heads) = 8 cores.
Each core computes, for its (batch b, head-group g):
  - qT/kT = wqk_g^T @ x_b^T            [512, L]   (head dims on partitions)
  - V     = x_b @ wv_g (+ ones cols)   [L, 2*193] (natural layout, per-head ones
                                                   column so the PV matmul also
                                                   produces softmax denominators)
  - S^T   = K Q^T per (k-block, q-tile), causal-trimmed, both heads of a
            pair row-packed into one concurrent PE pass; exp on ACT;
            multiplicative triangular mask (idle GpSimd) on diagonal blocks
  - out^T = V_ext^T @ E^T accumulated over k-blocks  -> PSUM
            (partition 64 resp. 32 holds the softmax denominator r)
  - attn^T = out^T * (1/r); 1/r on DVE (reciprocal_approx_fast, no ACT
            table swap); broadcast across partitions via a step-0-free-dim
            SBUF->SBUF DMA; DVE multiply
  - y_part = attn @ w_out[rows of g]   [L, 1024]  (row-parallel out-proj,
            bf16 partials summed on host)
Host gathers: y_b = sum_g y_part + (b_qkv_v @ w_out + b_out).

Scheduling: one global PSUM layout (scores 4 banks double-buffered,
attn-out/outproj 2 banks, projections 2 banks) so all phases coexist;
projection tiles and out-projection tiles are emitted as *filler* PE work
between attention k-blocks, keeping the PE warm while ACT streams exp.
All matmuls run in bf16 (inputs host-rounded).
"""

import sys
from collections import deque
from contextlib import ExitStack

if "/opt/trn_rl_repo" not in sys.path:
    sys.path.insert(0, "/opt/trn_rl_repo")

import ml_dtypes
import numpy as np

import concourse.bass as bass
import concourse.mybir as mybir
import concourse.tile as tile
from concourse import bacc
from concourse.ap import AP
from concourse.bass import ts
from concourse.bass_utils import run_bass_kernel_spmd

F32 = mybir.dt.float32
BF16 = mybir.dt.bfloat16
AF = mybir.ActivationFunctionType
OP = mybir.AluOpType

B, D, H = 2, 1024, 16
HD = 64           # head dim
NH = 4            # heads per core
GD = NH * HD      # 256 head dims per core
P = 128
QTW = 512         # q-tile width
VSTR = 193        # per-pair stride in the v tile: [V0(64)|1] + [z32|1|z31|V1(64)]
VW = 2 * VSTR     # v tile width (2 pairs)
SKEW = 4          # PV matmuls trail scores/exp by this many k-blocks


def bcast_ap(row_ap, n_part):
    """[1, N] SBUF AP -> (1, n_part, N) AP replicating the row (step-0 free
    dim), for DMA partition-broadcast."""
    from concourse.ap import AP

    dims = list(row_ap.ap)
    assert dims[0][1] == 1 and len(dims) == 2, dims
    return AP(row_ap.tensor, row_ap.offset,
              [list(dims[0]), [0, n_part], list(dims[1])])


def build_nc(L=2048):
    """Build the per-core Bass program. Same program for all 8 cores (SPMD)."""
    DK = D // P       # 8 contraction chunks
    LT = L // P       # l-tiles (16)
    QT = L // QTW     # q-tiles (4)
    QB = QTW // P     # k-blocks per q-tile (4)

    nc = bacc.Bacc("TRN2", target_bir_lowering=False, debug=False, num_devices=8)

    xT = nc.dram_tensor("xT", [D, L], BF16, kind="ExternalInput").ap()
    wqk = nc.dram_tensor("wqk", [D, 2 * GD], BF16, kind="ExternalInput").ap()
    wv = nc.dram_tensor("wv", [D, GD], BF16, kind="ExternalInput").ap()
    wo = nc.dram_tensor("wo", [GD, D], BF16, kind="ExternalInput").ap()
    bqk = nc.dram_tensor("bqk", [2 * GD, 1], F32, kind="ExternalInput").ap()
    mask = nc.dram_tensor("mask", [P, P], BF16, kind="ExternalInput").ap()
    # ones/zeros filler for the V slots: [1, 0*32, 1, 0*31] per partition
    vpat = nc.dram_tensor("vpat", [P, 65], BF16, kind="ExternalInput").ap()
    y = nc.dram_tensor("y", [L, D], BF16, kind="ExternalOutput").ap()

    with tile.TileContext(nc) as tc, ExitStack() as stk:
        # ---------- persistent SBUF pools ----------
        const = stk.enter_context(tc.tile_pool(name="const", bufs=1))
        qk_pool = stk.enter_context(tc.tile_pool(name="qk", bufs=1))
        v_pool = stk.enter_context(tc.tile_pool(name="v", bufs=1))
        attn_pool = stk.enter_context(tc.tile_pool(name="attn", bufs=1))
        wo_pool = stk.enter_context(tc.tile_pool(name="wo", bufs=1))
        xt_pool = stk.enter_context(tc.tile_pool(name="xt", bufs=1))
        wi_pool = stk.enter_context(tc.tile_pool(name="wi", bufs=1))
        e_pool = stk.enter_context(tc.tile_pool(name="e", bufs=5))
        r_pool = stk.enter_context(tc.tile_pool(name="rinv", bufs=2))
        bc_pool = stk.enter_context(tc.tile_pool(name="bc", bufs=2))
        y_pool = stk.enter_context(tc.tile_pool(name="ysb", bufs=3))

        # ---------- global PSUM layout (8 banks total) ----------
        # pss: scores, 2 tiles x [P,1024] f32 = 4 banks
        # pso: attention-out accumulators, 1 slot x [P,1024] f32 = 2 banks
        # psproj: projection accumulators, 2 x [P,512] f32 = 2 banks
        #   (closed after the last projection, banks reused for out-proj)
        pss = stk.enter_context(tc.tile_pool(name="pss", bufs=2, space="PSUM"))
        pso = stk.enter_context(tc.tile_pool(name="pso", bufs=1, space="PSUM"))
        psproj_cm = tc.tile_pool(name="pp", bufs=2, space="PSUM")
        psproj = psproj_cm.__enter__()

        # ---------- input DMAs (small consts first, then k-chunk trios) ----------
        bqk_sb = const.tile([P, 4], F32, tag="bqk", name="bqk_sb")
        nc.sync.dma_start(bqk_sb[:], bqk.rearrange("(m p) o -> p (m o)", p=P))
        mask_sb = const.tile([P, P], BF16, tag="mask", name="mask_sb")
        nc.sync.dma_start(mask_sb[:], mask)

        # batched input loads: a few large strided DMAs (issue cost ~0.7us
        # each on the Sync queue, so per-chunk loads serialize the startup).
        # Arrival order: qk weights for m=0,2 -> x -> wv -> m=1,3 -> wo, so
        # the first attention group's chain starts as early as possible.
        wqkm_sb = [wi_pool.tile([P, DK, P], BF16, tag=f"wqkm{m}",
                                name=f"wqkm_sb{m}") for m in range(4)]
        wv_all = wi_pool.tile([P, DK, GD], BF16, tag="wv", name="wv_all")
        xt_all = [xt_pool.tile([P, DK // 2, L], BF16, tag=f"xt{h}", name=f"xt_all{h}")
                  for h in range(2)]
        for m in (0, 2):
            nc.sync.dma_start(
                wqkm_sb[m][:],
                wqk[:, ts(m, P)].rearrange("(k p) c -> p k c", p=P))
        xv = xT.rearrange("(h k p) l -> h p k l", h=2, p=P)
        for h in range(2):
            nc.sync.dma_start(xt_all[h][:], xv[h])
        nc.sync.dma_start(wv_all[:], wv.rearrange("(k p) m -> p k m", p=P))
        for m in (1, 3):
            nc.sync.dma_start(
                wqkm_sb[m][:],
                wqk[:, ts(m, P)].rearrange("(k p) c -> p k c", p=P))
        wo_all = wo_pool.tile([P, 2, D], BF16, tag="wo", name="wo_all")
        nc.sync.dma_start(wo_all[:], wo.rearrange("(c p) d -> p c d", p=P))
        wo_sb = [wo_all[:, c, :] for c in range(2)]
        wv_sb = [wv_all[:, k, :] for k in range(DK)]
        xt_sb = [xt_all[k // 4][:, k % 4, :] for k in range(DK)]

        # m-tile 0,1 = qT (head pairs 01, 23); 2,3 = kT
        qk_sb = [qk_pool.tile([P, L], BF16, tag=f"qk{m}", name=f"qk_sb{m}") for m in range(4)]
        v_sb = [v_pool.tile([P, VW], BF16, tag=f"v{t}", name=f"v_sb{t}") for t in range(LT)]
        attn_sb = [attn_pool.tile([P, L], BF16, tag=f"attn{p}", name=f"attn_sb{p}") for p in range(2)]

        # ---------- emission units ----------
        def emit_qk(m, n):
            """qT/kT [P,512] slice: [m-tile, n-slice] = wqk^T @ xT + bias."""
            ps = psproj.tile([P, QTW], F32, tag="pp", name="ps_p")
            for k in range(DK):
                nc.tensor.matmul(
                    ps[:],
                    wqkm_sb[m][:, k, :],
                    xt_sb[k][:, ts(n, QTW)],
                    start=(k == 0),
                    stop=(k == DK - 1),
                )
            nc.vector.tensor_scalar(
                out=qk_sb[m][:, ts(n, QTW)],
                in0=ps[:],
                scalar1=bqk_sb[:, m : m + 1],
                scalar2=None,
                op0=OP.add,
            )

        def emit_v(lt):
            """V natural [128, 256] for l-tile lt, packed into [V|ones] slots."""
            vt = v_sb[lt]
            vv = vt[:, 0:VW].rearrange("p (a c) -> p a c", a=2, c=VSTR)
            vp3 = AP(vpat.tensor, vpat.offset,
                     [list(vpat.ap[0]), [0, 2], list(vpat.ap[1])])
            nc.sync.dma_start(vv[:, :, 64:129], vp3)
            ps = psproj.tile([P, GD], F32, tag="pp", name="ps_v")
            for k in range(DK):
                nc.tensor.matmul(
                    ps[:],
                    xt_sb[k][:, ts(lt, P)],
                    wv_sb[k],
                    start=(k == 0),
                    stop=(k == DK - 1),
                )
            pv = ps[:].rearrange("p (a c) -> p a c", a=2, c=2 * HD)
            nc.vector.tensor_copy(vv[:, :, 0:64], pv[:, :, 0:64])       # heads 0,2
            nc.vector.tensor_copy(vv[:, :, 129:193], pv[:, :, 64:128])  # heads 1,3

        norm_done = set()  # (pair, qt) whose normalize has been emitted

        def emit_out(lt4, opool):
            """Out-proj for l-tile lt4 (global index): y[lt4] = attn @ wo.
            Drains the pend queue until both attn tiles for this q-tile have
            been written (their normalize emitted) — else the lhsT read
            would precede the write in program order and read stale data."""
            qt = lt4 // QB
            while not ((0, qt) in norm_done and (1, qt) in norm_done):
                pend.popleft()()
            py = opool.tile([P, 2 * QTW], F32, tag="po", name="ps_y")
            for c in range(2):
                for nh in range(2):
                    nc.tensor.matmul(
                        py[:, ts(nh, QTW)],
                        attn_sb[c][:, ts(lt4, P)],
                        wo_sb[c][:, ts(nh, QTW)],
                        start=(c == 0),
                        stop=(c == 1),
                    )
            yt = y_pool.tile([P, 2 * QTW], BF16, tag="y", name="y_t")
            nc.vector.tensor_copy(yt[:], py[:])
            nc.sync.dma_start(y[ts(lt4, P)], yt[:])

        filler = deque()

        def emit_filler(n=1):
            for _ in range(n):
                if filler:
                    f = filler.popleft()
                    if f is not None:  # None = deliberate empty slot
                        f()

        def vext(vt, pair, hl):
            base = VSTR * pair
            if hl == 0:
                return vt[:, base : base + 65]       # M=65: V at 0-63, r at 64
            return vt[:, base + 65 : base + VSTR]    # M=128: ones@32, V at 64-127

        # global deferred-work queue: PV matmuls trail their scores/exp by
        # SKEW blocks ACROSS group boundaries, so the tail of one group's
        # pipeline drains under the next group's score stream.
        pend = deque()

        def emit_attn(pair, qt, opool):
            """Attention group: scores+exp+PV over k-blocks, then normalize."""
            q_t = qk_sb[pair]
            k_t = qk_sb[2 + pair]
            # h0 -> cols 0:512 (parts 0-64), h1 -> cols 512:1024 (parts 32-127)
            out_ps = opool.tile([P, 2 * QTW], F32, tag="po", name="ps_o")
            nblk = QB * qt + QB

            def front(j):
                """Row-packed scores + exp (+ mask) for k-block j.
                Returns a closure emitting the two PV matmuls."""
                sp = pss.tile([P, 2 * QTW], F32, tag="pss", name="ps_s")
                e_t = e_pool.tile([P, 2 * QTW], BF16, tag="e", name="e_t")
                diag = j >= QB * qt
                da = (j - QB * qt) * P if diag else 0
                for hl in range(2):
                    hb = 64 * hl
                    nc.tensor.matmul(
                        sp[:, hl * QTW + da : (hl + 1) * QTW],
                        k_t[hb : hb + 64, ts(j, P)],
                        q_t[hb : hb + 64, qt * QTW + da : (qt + 1) * QTW],
                        start=True, stop=True)
                if da == 0:
                    nc.scalar.activation(e_t[:], sp[:], AF.Exp, scale=0.125)
                else:
                    nc.scalar.activation(e_t[:, da:QTW], sp[:, da:QTW],
                                         AF.Exp, scale=0.125)
                    nc.scalar.activation(
                        e_t[:, QTW + da : 2 * QTW],
                        sp[:, QTW + da : 2 * QTW], AF.Exp, scale=0.125)
                if diag:  # triangular masks on idle GpSimd
                    nc.gpsimd.tensor_tensor(
                        out=e_t[:, da : da + P],
                        in0=e_t[:, da : da + P],
                        in1=mask_sb[:], op=OP.mult)
                    nc.gpsimd.tensor_tensor(
                        out=e_t[:, QTW + da : QTW + da + P],
                        in0=e_t[:, QTW + da : QTW + da + P],
                        in1=mask_sb[:], op=OP.mult)

                def emit_pv(j=j, da=da, e_t=e_t):
                    for hl in range(2):
                        if hl == 0:
                            out = out_ps[0:65, da:QTW]
                        else:
                            out = out_ps[:, QTW + da : 2 * QTW]
                        nc.tensor.matmul(
                            out, vext(v_sb[j], pair, hl),
                            e_t[:, hl * QTW + da : (hl + 1) * QTW],
                            start=(j == 0), stop=(j == nblk - 1))
                return emit_pv

            for j in range(nblk):
                pend.append(front(j))
                while len(pend) > SKEW:
                    pend.popleft()()
                emit_filler(1)

            def normalize():
                # copy r rows PSUM->SBUF, broadcast across partitions via
                # DMA, one full-tile DVE reciprocal, eviction multiplies
                rt = r_pool.tile([P, QTW], F32, tag="rinv", name="rinv_t")
                bc = bc_pool.tile([P, QTW], F32, tag="bc", name="bc_t")
                rb = bc_pool.tile([P, QTW], F32, tag="rb", name="rb_t")
                nc.vector.tensor_copy(rt[64:65, :], out_ps[64:65, 0:QTW])
                nc.vector.tensor_copy(
                    rt[32:33, :], out_ps[32:33, QTW : 2 * QTW])
                nc.scalar.dma_start(bc[0:64, :], bcast_ap(rt[64:65, :], 64))
                nc.scalar.dma_start(bc[64:P, :], bcast_ap(rt[32:33, :], 64))
                nc.vector.reciprocal_approx_fast(out=rb[:], in_=bc[:])
                nc.vector.tensor_tensor(
                    out=attn_sb[pair][0:64, ts(qt, QTW)],
                    in0=out_ps[0:64, 0:QTW], in1=rb[0:64, :], op=OP.mult)
                nc.vector.tensor_tensor(
                    out=attn_sb[pair][64:P, ts(qt, QTW)],
                    in0=out_ps[64:P, QTW : 2 * QTW], in1=rb[64:P, :],
                    op=OP.mult)
                norm_done.add((pair, qt))

            pend.append(normalize)

        # ---------- emission schedule ----------
        # pre-attention: just enough projection for A(0,0)
        emit_qk(0, 0)
        emit_qk(2, 0)
        for lt in range(4):
            emit_v(lt)

        # filler units threaded into the attention block stream
        filler.extend([
            lambda: emit_qk(1, 0), lambda: emit_qk(3, 0),
            lambda: emit_v(4), lambda: emit_v(5),
        ])
        emit_attn(0, 0, pso)
        filler.extend([
            lambda: emit_qk(0, 1), lambda: emit_qk(2, 1),
            lambda: emit_v(6), lambda: emit_v(7),
        ])
        emit_attn(1, 0, pso)
        filler.extend([
            lambda: emit_qk(1, 1), lambda: emit_qk(3, 1),
            lambda: emit_v(8), lambda: emit_v(9),
            lambda: emit_qk(0, 2), lambda: emit_qk(2, 2),
            lambda: emit_v(10), lambda: emit_v(11),
        ])
        emit_attn(0, 1, pso)
        filler.extend([
            lambda: emit_qk(1, 2), lambda: emit_qk(3, 2),
            lambda: emit_v(12), lambda: emit_v(13),
            lambda: emit_qk(0, 3), lambda: emit_qk(2, 3),
            lambda: emit_qk(1, 3), lambda: emit_qk(3, 3),
        ])
        emit_attn(1, 1, pso)
        emit_v(14)
        emit_v(15)
        # projection PSUM banks free -> second attention-out pool so that
        # consecutive groups alternate banks (no normalize serialization)
        psproj_cm.__exit__(None, None, None)
        psoL_cm = tc.tile_pool(name="psoL", bufs=1, space="PSUM")
        psoL = psoL_cm.__enter__()
        filler.extend([
            None, None, None, None,  # let the A(1,1) normalize chain execute
            lambda: emit_out(0, pso), lambda: emit_out(1, pso),
            lambda: emit_out(2, pso), lambda: emit_out(3, pso),
            lambda: emit_out(4, pso), lambda: emit_out(5, pso),
            lambda: emit_out(6, pso), lambda: emit_out(7, pso),
        ])
        emit_attn(0, 2, psoL)
        emit_attn(1, 2, pso)
        filler.extend([
            None, None, None, None,  # let the A(1,2) normalize chain execute
            lambda: emit_out(8, pso), lambda: emit_out(9, pso),
            lambda: emit_out(10, pso), lambda: emit_out(11, pso),
        ])
        emit_attn(0, 3, psoL)
        emit_attn(1, 3, pso)
        while pend:
            pend.popleft()()
        emit_filler(len(filler))
        emit_out(12, psoL)
        emit_out(13, pso)
        emit_out(14, psoL)
        emit_out(15, pso)
        psoL_cm.__exit__(None, None, None)

    nc.compile()
    return nc


def make_mask():
    return (np.arange(P)[:, None] <= np.arange(P)[None, :]).astype(
        ml_dtypes.bfloat16)


def make_vpat():
    pat = np.zeros((P, 65), ml_dtypes.bfloat16)
    pat[:, 0] = 1.0   # even-head ones col (tile col 64): r -> partition 64
    pat[:, 33] = 1.0  # odd-head ones col (tile col 97): r -> partition 32
    return pat


def shard_inputs(x, w_qkv, b_qkv, w_out, L=2048):
    """Host-side sharding: core c = (batch c//4, head-group c%4)."""
    x = np.asarray(x, np.float32)
    w_qkv = np.asarray(w_qkv, np.float32)
    b_qkv = np.asarray(b_qkv, np.float32)
    w_out = np.asarray(w_out, np.float32)
    mask = make_mask()
    xTs = [np.ascontiguousarray(x[b].T.astype(ml_dtypes.bfloat16))
           for b in range(B)]
    in_maps = []
    for c in range(8):
        b, g = divmod(c, 4)
        qs, ks, vs = 256 * g, D + 256 * g, 2 * D + 256 * g
        wqk = np.ascontiguousarray(
            np.concatenate(
                [w_qkv[:, qs : qs + GD], w_qkv[:, ks : ks + GD]], axis=1
            ).astype(ml_dtypes.bfloat16)
        )
        wv = np.ascontiguousarray(
            w_qkv[:, vs : vs + GD].astype(ml_dtypes.bfloat16))
        wo = np.ascontiguousarray(
            w_out[256 * g : 256 * g + GD, :].astype(ml_dtypes.bfloat16))
        bqk = np.concatenate(
            [b_qkv[qs : qs + GD], b_qkv[ks : ks + GD]]
        ).reshape(2 * GD, 1).astype(np.float32)
        in_maps.append(
            {"xT": xTs[b], "wqk": wqk, "wv": wv, "wo": wo, "bqk": bqk,
             "mask": mask, "vpat": make_vpat()}
        )
    return in_maps


_NC_CACHE = {}


def get_nc(L=2048):
    if L not in _NC_CACHE:
        _NC_CACHE[L] = build_nc(L)
    return _NC_CACHE[L]


def gather(results, b_qkv, w_out, b_out, L=2048):
    fix = (np.asarray(b_qkv, np.float32)[2 * D :] @ np.asarray(w_out, np.float32)
           + np.asarray(b_out, np.float32))
    y = np.zeros((B, L, D), np.float32)
    for c in range(8):
        b = c // 4
        y[b] += np.asarray(results[c]["y"], dtype=np.float32)
    y += fix[None, None, :]
    return y


def kernel(x, w_qkv, b_qkv, w_out, b_out):
    L = x.shape[1]
    nc = get_nc(L)
    in_maps = shard_inputs(x, w_qkv, b_qkv, w_out, L=L)
    res = run_bass_kernel_spmd(nc, in_maps, core_ids=list(range(8)))
    return gather(res.results, b_qkv, w_out, b_out, L=L)


# revision 39
# speedup vs baseline: 1.0363x; 1.0363x over previous
"""Causal multi-head attention (B=2, L=2048, D=1024, H=16) on 8 trn2 cores.

Sharding: DP on batch (2) x TP on heads (4 groups of 4 heads) = 8 cores.
Each core computes, for its (batch b, head-group g):
  - qT/kT = wqk_g^T @ x_b^T            [512, L]   (head dims on partitions)
  - V     = x_b @ wv_g (+ ones cols)   [L, 2*193] (natural layout, per-head ones
                                                   column so the PV matmul also
                                                   produces softmax denominators)
  - S^T   = K Q^T per (k-block, q-tile), causal-trimmed, both heads of a
            pair row-packed into one concurrent PE pass; exp on ACT;
            multiplicative triangular mask (idle GpSimd) on diagonal blocks
  - out^T = V_ext^T @ E^T accumulated over k-blocks  -> PSUM
            (partition 64 resp. 32 holds the softmax denominator r)
  - attn^T = out^T * (1/r); 1/r on DVE (reciprocal_approx_fast, no ACT
            table swap); broadcast across partitions via a step-0-free-dim
            SBUF->SBUF DMA; DVE multiply
  - y_part = attn @ w_out[rows of g]   [L, 1024]  (row-parallel out-proj,
            bf16 partials summed on host)
Host gathers: y_b = sum_g y_part + (b_qkv_v @ w_out + b_out).

Scheduling: one global PSUM layout (scores 4 banks double-buffered,
attn-out/outproj 2 banks, projections 2 banks) so all phases coexist;
projection tiles and out-projection tiles are emitted as *filler* PE work
between attention k-blocks, keeping the PE warm while ACT streams exp.
All matmuls run in bf16 (inputs host-rounded).
"""

import sys
from collections import deque
from contextlib import ExitStack

if "/opt/trn_rl_repo" not in sys.path:
    sys.path.insert(0, "/opt/trn_rl_repo")

import ml_dtypes
import numpy as np

import concourse.bass as bass
import concourse.mybir as mybir
import concourse.tile as tile
from concourse import bacc
from concourse.ap import AP
from concourse.bass import ts
from concourse.bass_utils import run_bass_kernel_spmd

F32 = mybir.dt.float32
BF16 = mybir.dt.bfloat16
AF = mybir.ActivationFunctionType
OP = mybir.AluOpType

B, D, H = 2, 1024, 16
HD = 64           # head dim
NH = 4            # heads per core
GD = NH * HD      # 256 head dims per core
P = 128
QTW = 512         # q-tile width
VSTR = 193        # per-pair stride in the v tile: [V0(64)|1] + [z32|1|z31|V1(64)]
VW = 2 * VSTR     # v tile width (2 pairs)
SKEW = 4          # PV matmuls trail scores/exp by this many k-blocks


def bcast_ap(row_ap, n_part):
    """[1, N] SBUF AP -> (1, n_part, N) AP replicating the row (step-0 free
    dim), for DMA partition-broadcast."""
    from concourse.ap import AP

    dims = list(row_ap.ap)
    assert dims[0][1] == 1 and len(dims) == 2, dims
    return AP(row_ap.tensor, row_ap.offset,
              [list(dims[0]), [0, n_part], list(dims[1])])


def build_nc(L=2048):
    """Build the per-core Bass program. Same program for all 8 cores (SPMD)."""
    DK = D // P       # 8 contraction chunks
    LT = L // P       # l-tiles (16)
    QT = L // QTW     # q-tiles (4)
    QB = QTW // P     # k-blocks per q-tile (4)

    nc = bacc.Bacc("TRN2", target_bir_lowering=False, debug=False, num_devices=8)

    xT = nc.dram_tensor("xT", [D, L], BF16, kind="ExternalInput").ap()
    wqk = nc.dram_tensor("wqk", [D, 2 * GD], BF16, kind="ExternalInput").ap()
    wv = nc.dram_tensor("wv", [D, GD], BF16, kind="ExternalInput").ap()
    wo = nc.dram_tensor("wo", [GD, D], BF16, kind="ExternalInput").ap()
    bqk = nc.dram_tensor("bqk", [2 * GD, 1], F32, kind="ExternalInput").ap()
    mask = nc.dram_tensor("mask", [P, P], BF16, kind="ExternalInput").ap()
    # ones/zeros filler for the V slots: [1, 0*32, 1, 0*31] per partition
    vpat = nc.dram_tensor("vpat", [P, 65], BF16, kind="ExternalInput").ap()
    y = nc.dram_tensor("y", [L, D], BF16, kind="ExternalOutput").ap()

    with tile.TileContext(nc) as tc, ExitStack() as stk:
        # ---------- persistent SBUF pools ----------
        const = stk.enter_context(tc.tile_pool(name="const", bufs=1))
        qk_pool = stk.enter_context(tc.tile_pool(name="qk", bufs=1))
        v_pool = stk.enter_context(tc.tile_pool(name="v", bufs=1))
        attn_pool = stk.enter_context(tc.tile_pool(name="attn", bufs=1))
        wo_pool = stk.enter_context(tc.tile_pool(name="wo", bufs=1))
        xt_pool = stk.enter_context(tc.tile_pool(name="xt", bufs=1))
        wi_pool = stk.enter_context(tc.tile_pool(name="wi", bufs=1))
        e_pool = stk.enter_context(tc.tile_pool(name="e", bufs=5))
        r_pool = stk.enter_context(tc.tile_pool(name="rinv", bufs=2))
        bc_pool = stk.enter_context(tc.tile_pool(name="bc", bufs=2))
        y_pool = stk.enter_context(tc.tile_pool(name="ysb", bufs=3))

        # ---------- global PSUM layout (8 banks total) ----------
        # pss: scores, 2 tiles x [P,1024] f32 = 4 banks
        # pso: attention-out accumulators, 1 slot x [P,1024] f32 = 2 banks
        # psproj: projection accumulators, 2 x [P,512] f32 = 2 banks
        #   (closed after the last projection, banks reused for out-proj)
        pss = stk.enter_context(tc.tile_pool(name="pss", bufs=2, space="PSUM"))
        pso = stk.enter_context(tc.tile_pool(name="pso", bufs=1, space="PSUM"))
        psproj_cm = tc.tile_pool(name="pp", bufs=2, space="PSUM")
        psproj = psproj_cm.__enter__()

        # ---------- input DMAs (small consts first, then k-chunk trios) ----------
        bqk_sb = const.tile([P, 4], F32, tag="bqk", name="bqk_sb")
        nc.sync.dma_start(bqk_sb[:], bqk.rearrange("(m p) o -> p (m o)", p=P))
        mask_sb = const.tile([P, P], BF16, tag="mask", name="mask_sb")
        nc.sync.dma_start(mask_sb[:], mask)

        # batched input loads: a few large strided DMAs (issue cost ~0.7us
        # each on the Sync queue, so per-chunk loads serialize the startup).
        # Arrival order: qk weights for m=0,2 -> x -> wv -> m=1,3 -> wo, so
        # the first attention group's chain starts as early as possible.
        wqkm_sb = [wi_pool.tile([P, DK, P], BF16, tag=f"wqkm{m}",
                                name=f"wqkm_sb{m}") for m in range(4)]
        wv_all = wi_pool.tile([P, DK, GD], BF16, tag="wv", name="wv_all")
        xt_all = [xt_pool.tile([P, DK // 2, L], BF16, tag=f"xt{h}", name=f"xt_all{h}")
                  for h in range(2)]
        for m in (0, 2):
            nc.sync.dma_start(
                wqkm_sb[m][:],
                wqk[:, ts(m, P)].rearrange("(k p) c -> p k c", p=P))
        xv = xT.rearrange("(h k p) l -> h p k l", h=2, p=P)
        for h in range(2):
            nc.sync.dma_start(xt_all[h][:], xv[h])
        nc.sync.dma_start(wv_all[:], wv.rearrange("(k p) m -> p k m", p=P))
        for m in (1, 3):
            nc.sync.dma_start(
                wqkm_sb[m][:],
                wqk[:, ts(m, P)].rearrange("(k p) c -> p k c", p=P))
        wo_all = wo_pool.tile([P, 2, D], BF16, tag="wo", name="wo_all")
        nc.sync.dma_start(wo_all[:], wo.rearrange("(c p) d -> p c d", p=P))
        wo_sb = [wo_all[:, c, :] for c in range(2)]
        wv_sb = [wv_all[:, k, :] for k in range(DK)]
        xt_sb = [xt_all[k // 4][:, k % 4, :] for k in range(DK)]

        # m-tile 0,1 = qT (head pairs 01, 23); 2,3 = kT
        qk_sb = [qk_pool.tile([P, L], BF16, tag=f"qk{m}", name=f"qk_sb{m}") for m in range(4)]
        v_sb = [v_pool.tile([P, VW], BF16, tag=f"v{t}", name=f"v_sb{t}") for t in range(LT)]
        attn_sb = [attn_pool.tile([P, L], BF16, tag=f"attn{p}", name=f"attn_sb{p}") for p in range(2)]

        # ---------- emission units ----------
        def emit_qk(m, n):
            """qT/kT [P,512] slice: [m-tile, n-slice] = wqk^T @ xT + bias."""
            ps = psproj.tile([P, QTW], F32, tag="pp", name="ps_p")
            for k in range(DK):
                nc.tensor.matmul(
                    ps[:],
                    wqkm_sb[m][:, k, :],
                    xt_sb[k][:, ts(n, QTW)],
                    start=(k == 0),
                    stop=(k == DK - 1),
                )
            nc.vector.tensor_scalar(
                out=qk_sb[m][:, ts(n, QTW)],
                in0=ps[:],
                scalar1=bqk_sb[:, m : m + 1],
                scalar2=None,
                op0=OP.add,
            )

        def emit_v(lt):
            """V natural [128, 256] for l-tile lt, packed into [V|ones] slots."""
            vt = v_sb[lt]
            vv = vt[:, 0:VW].rearrange("p (a c) -> p a c", a=2, c=VSTR)
            vp3 = AP(vpat.tensor, vpat.offset,
                     [list(vpat.ap[0]), [0, 2], list(vpat.ap[1])])
            nc.sync.dma_start(vv[:, :, 64:129], vp3)
            ps = psproj.tile([P, GD], F32, tag="pp", name="ps_v")
            for k in range(DK):
                nc.tensor.matmul(
                    ps[:],
                    xt_sb[k][:, ts(lt, P)],
                    wv_sb[k],
                    start=(k == 0),
                    stop=(k == DK - 1),
                )
            pv = ps[:].rearrange("p (a c) -> p a c", a=2, c=2 * HD)
            nc.vector.tensor_copy(vv[:, :, 0:64], pv[:, :, 0:64])       # heads 0,2
            nc.vector.tensor_copy(vv[:, :, 129:193], pv[:, :, 64:128])  # heads 1,3

        norm_done = set()  # (pair, qt) whose normalize has been emitted

        def emit_out(lt4, opool):
            """Out-proj for l-tile lt4 (global index): y[lt4] = attn @ wo.
            Drains the pend queue until both attn tiles for this q-tile have
            been written (their normalize emitted) — else the lhsT read
            would precede the write in program order and read stale data."""
            qt = lt4 // QB
            while not ((0, qt) in norm_done and (1, qt) in norm_done):
                pend.popleft()()
            py = opool.tile([P, 2 * QTW], F32, tag="po", name="ps_y")
            for c in range(2):
                for nh in range(2):
                    nc.tensor.matmul(
                        py[:, ts(nh, QTW)],
                        attn_sb[c][:, ts(lt4, P)],
                        wo_sb[c][:, ts(nh, QTW)],
                        start=(c == 0),
                        stop=(c == 1),
                    )
            yt = y_pool.tile([P, 2 * QTW], BF16, tag="y", name="y_t")
            nc.scalar.copy(yt[:], py[:])  # ACT, keeps the DVE FIFO short
            nc.sync.dma_start(y[ts(lt4, P)], yt[:])

        filler = deque()

        def emit_filler(n=1):
            for _ in range(n):
                if filler:
                    f = filler.popleft()
                    if f is not None:  # None = deliberate empty slot
                        f()

        def vext(vt, pair, hl):
            base = VSTR * pair
            if hl == 0:
                return vt[:, base : base + 65]       # M=65: V at 0-63, r at 64
            return vt[:, base + 65 : base + VSTR]    # M=128: ones@32, V at 64-127

        # global deferred-work queue: PV matmuls trail their scores/exp by
        # SKEW blocks ACROSS group boundaries, so the tail of one group's
        # pipeline drains under the next group's score stream.
        pend = deque()

        def emit_attn(pair, qt, opool):
            """Attention group: scores+exp+PV over k-blocks, then normalize."""
            q_t = qk_sb[pair]
            k_t = qk_sb[2 + pair]
            # h0 -> cols 0:512 (parts 0-64), h1 -> cols 512:1024 (parts 32-127)
            out_ps = opool.tile([P, 2 * QTW], F32, tag="po", name="ps_o")
            nblk = QB * qt + QB

            def front(j):
                """Row-packed scores + exp (+ mask) for k-block j.
                Returns a closure emitting the two PV matmuls."""
                sp = pss.tile([P, 2 * QTW], F32, tag="pss", name="ps_s")
                e_t = e_pool.tile([P, 2 * QTW], BF16, tag="e", name="e_t")
                diag = j >= QB * qt
                da = (j - QB * qt) * P if diag else 0
                for hl in range(2):
                    hb = 64 * hl
                    nc.tensor.matmul(
                        sp[:, hl * QTW + da : (hl + 1) * QTW],
                        k_t[hb : hb + 64, ts(j, P)],
                        q_t[hb : hb + 64, qt * QTW + da : (qt + 1) * QTW],
                        start=True, stop=True)
                if da == 0:
                    nc.scalar.activation(e_t[:], sp[:], AF.Exp, scale=0.125)
                else:
                    # one strided ACTIVATE over both heads' trimmed ranges
                    sv = sp[:].rearrange("p (a c) -> p a c", a=2, c=QTW)
                    ev = e_t[:].rearrange("p (a c) -> p a c", a=2, c=QTW)
                    nc.scalar.activation(ev[:, :, da:QTW], sv[:, :, da:QTW],
                                         AF.Exp, scale=0.125)
                if diag:  # triangular masks on idle GpSimd
                    nc.gpsimd.tensor_tensor(
                        out=e_t[:, da : da + P],
                        in0=e_t[:, da : da + P],
                        in1=mask_sb[:], op=OP.mult)
                    nc.gpsimd.tensor_tensor(
                        out=e_t[:, QTW + da : QTW + da + P],
                        in0=e_t[:, QTW + da : QTW + da + P],
                        in1=mask_sb[:], op=OP.mult)

                def emit_pv(j=j, da=da, e_t=e_t):
                    for hl in range(2):
                        if hl == 0:
                            out = out_ps[0:65, da:QTW]
                        else:
                            out = out_ps[:, QTW + da : 2 * QTW]
                        nc.tensor.matmul(
                            out, vext(v_sb[j], pair, hl),
                            e_t[:, hl * QTW + da : (hl + 1) * QTW],
                            start=(j == 0), stop=(j == nblk - 1))
                return emit_pv

            for j in range(nblk):
                pend.append(front(j))
                while len(pend) > SKEW:
                    pend.popleft()()
                emit_filler(1)

            def normalize():
                # copy r rows PSUM->SBUF, broadcast across partitions via
                # DMA, one full-tile DVE reciprocal, eviction multiplies
                rt = r_pool.tile([P, QTW], F32, tag="rinv", name="rinv_t")
                bc = bc_pool.tile([P, QTW], F32, tag="bc", name="bc_t")
                rb = bc_pool.tile([P, QTW], F32, tag="rb", name="rb_t")
                nc.vector.tensor_copy(rt[64:65, :], out_ps[64:65, 0:QTW])
                nc.vector.tensor_copy(
                    rt[32:33, :], out_ps[32:33, QTW : 2 * QTW])
                nc.scalar.dma_start(bc[0:64, :], bcast_ap(rt[64:65, :], 64))
                nc.scalar.dma_start(bc[64:P, :], bcast_ap(rt[32:33, :], 64))
                nc.vector.reciprocal_approx_fast(out=rb[:], in_=bc[:])
                nc.vector.tensor_tensor(
                    out=attn_sb[pair][0:64, ts(qt, QTW)],
                    in0=out_ps[0:64, 0:QTW], in1=rb[0:64, :], op=OP.mult)
                nc.vector.tensor_tensor(
                    out=attn_sb[pair][64:P, ts(qt, QTW)],
                    in0=out_ps[64:P, QTW : 2 * QTW], in1=rb[64:P, :],
                    op=OP.mult)
                norm_done.add((pair, qt))

            pend.append(normalize)

        # ---------- emission schedule ----------
        # pre-attention: just enough projection for A(0,0)
        emit_qk(0, 0)
        emit_qk(2, 0)
        for lt in range(4):
            emit_v(lt)

        # filler units threaded into the attention block stream
        filler.extend([
            lambda: emit_qk(1, 0), lambda: emit_qk(3, 0),
            lambda: emit_v(4), lambda: emit_v(5),
        ])
        emit_attn(0, 0, pso)
        filler.extend([
            lambda: emit_qk(0, 1), lambda: emit_qk(2, 1),
            lambda: emit_v(6), lambda: emit_v(7),
        ])
        emit_attn(1, 0, pso)
        filler.extend([
            lambda: emit_qk(1, 1), lambda: emit_qk(3, 1),
            lambda: emit_v(8), lambda: emit_v(9),
            lambda: emit_qk(0, 2), lambda: emit_qk(2, 2),
            lambda: emit_v(10), lambda: emit_v(11),
        ])
        emit_attn(0, 1, pso)
        filler.extend([
            lambda: emit_qk(1, 2), lambda: emit_qk(3, 2),
            lambda: emit_v(12), lambda: emit_v(13),
            lambda: emit_qk(0, 3), lambda: emit_qk(2, 3),
            lambda: emit_qk(1, 3), lambda: emit_qk(3, 3),
        ])
        emit_attn(1, 1, pso)
        emit_v(14)
        emit_v(15)
        # projection PSUM banks free -> second attention-out pool so that
        # consecutive groups alternate banks (no normalize serialization)
        psproj_cm.__exit__(None, None, None)
        psoL_cm = tc.tile_pool(name="psoL", bufs=1, space="PSUM")
        psoL = psoL_cm.__enter__()
        filler.extend([
            None, None, None, None, None, None,  # let A(1,1) normalize run
            lambda: emit_out(0, pso), lambda: emit_out(1, pso),
            lambda: emit_out(2, pso), lambda: emit_out(3, pso),
            lambda: emit_out(4, pso), lambda: emit_out(5, pso),
            lambda: emit_out(6, pso), lambda: emit_out(7, pso),
        ])
        emit_attn(0, 2, psoL)
        emit_attn(1, 2, pso)
        filler.extend([
            None, None, None, None, None, None,  # let A(1,2) normalize run
            lambda: emit_out(8, pso), lambda: emit_out(9, pso),
            lambda: emit_out(10, pso), lambda: emit_out(11, pso),
        ])
        emit_attn(0, 3, psoL)
        emit_attn(1, 3, pso)
        while pend:
            pend.popleft()()
        emit_filler(len(filler))
        emit_out(12, psoL)
        emit_out(13, pso)
        emit_out(14, psoL)
        emit_out(15, pso)
        psoL_cm.__exit__(None, None, None)

    nc.compile()
    return nc


def make_mask():
    return (np.arange(P)[:, None] <= np.arange(P)[None, :]).astype(
        ml_dtypes.bfloat16)


def make_vpat():
    pat = np.zeros((P, 65), ml_dtypes.bfloat16)
    pat[:, 0] = 1.0   # even-head ones col (tile col 64): r -> partition 64
    pat[:, 33] = 1.0  # odd-head ones col (tile col 97): r -> partition 32
    return pat


def shard_inputs(x, w_qkv, b_qkv, w_out, L=2048):
    """Host-side sharding: core c = (batch c//4, head-group c%4)."""
    x = np.asarray(x, np.float32)
    w_qkv = np.asarray(w_qkv, np.float32)
    b_qkv = np.asarray(b_qkv, np.float32)
    w_out = np.asarray(w_out, np.float32)
    mask = make_mask()
    xTs = [np.ascontiguousarray(x[b].T.astype(ml_dtypes.bfloat16))
           for b in range(B)]
    in_maps = []
    for c in range(8):
        b, g = divmod(c, 4)
        qs, ks, vs = 256 * g, D + 256 * g, 2 * D + 256 * g
        wqk = np.ascontiguousarray(
            np.concatenate(
                [w_qkv[:, qs : qs + GD], w_qkv[:, ks : ks + GD]], axis=1
            ).astype(ml_dtypes.bfloat16)
        )
        wv = np.ascontiguousarray(
            w_qkv[:, vs : vs + GD].astype(ml_dtypes.bfloat16))
        wo = np.ascontiguousarray(
            w_out[256 * g : 256 * g + GD, :].astype(ml_dtypes.bfloat16))
        bqk = np.concatenate(
            [b_qkv[qs : qs + GD], b_qkv[ks : ks + GD]]
        ).reshape(2 * GD, 1).astype(np.float32)
        in_maps.append(
            {"xT": xTs[b], "wqk": wqk, "wv": wv, "wo": wo, "bqk": bqk,
             "mask": mask, "vpat": make_vpat()}
        )
    return in_maps


_NC_CACHE = {}


def get_nc(L=2048):
    if L not in _NC_CACHE:
        _NC_CACHE[L] = build_nc(L)
    return _NC_CACHE[L]


def gather(results, b_qkv, w_out, b_out, L=2048):
    fix = (np.asarray(b_qkv, np.float32)[2 * D :] @ np.asarray(w_out, np.float32)
           + np.asarray(b_out, np.float32))
    y = np.zeros((B, L, D), np.float32)
    for c in range(8):
        b = c // 4
        y[b] += np.asarray(results[c]["y"], dtype=np.float32)
    y += fix[None, None, :]
    return y


def kernel(x, w_qkv, b_qkv, w_out, b_out):
    L = x.shape[1]
    nc = get_nc(L)
    in_maps = shard_inputs(x, w_qkv, b_qkv, w_out, L=L)
    res = run_bass_kernel_spmd(nc, in_maps, core_ids=list(range(8)))
    return gather(res.results, b_qkv, w_out, b_out, L=L)
